# revision 1
# baseline (speedup 1.0000x reference)
"""Trainium2 Bass kernel for a dense transformer encoder layer.

Reference computation (per batch element b, fp32):
    Q = src @ Wq + bq ; K = src @ Wk + bk ; V = src @ Wv + bv
    attn = softmax(Q @ K.T)          (no 1/sqrt(d) scaling)
    x    = LN(src + (attn @ V) @ Wo + bo) * g1 + be1
    out  = LN(x + relu(x @ W1 + b1) @ W2 + b2) * g2 + be2

Sharding: 8 cores, sequence-parallel with duplicated KV.  Core c handles
batch element b = c//2, query half h = c%2 (1024 query rows each).  Each
core computes K/V for its full batch element (2048 tokens), so there are
no collectives; K-projection bias bk is dropped entirely (softmax rows are
shift-invariant and Q·bk is constant along the softmax axis).

All activations are kept feature-major in SBUF ([features-on-partitions,
tokens-on-free]) so every projection chains on the PE without transposes;
only the attention probabilities are transposed (PE transpose fused with
the softmax 1/sum normalization via a diagonal rhs).  Matmuls run in
float32r (1 elem/cycle on TRN2 at N>=256); V and attn-probs are stored
bf16 (their error is averaged down by the softmax-weighted sum).
"""

import numpy as np

import concourse.bass as bass
import concourse.mybir as mybir
import concourse.tile as tile
from concourse import bacc
from concourse.masks import make_identity

FP = mybir.dt.float32
FR = mybir.dt.float32r
BF = mybir.dt.bfloat16
AX = mybir.AxisListType
ALU = mybir.AluOpType
ACTF = mybir.ActivationFunctionType
P = 128

EPS = 1e-5


def _r(ap):
    """View an fp32 AP as float32r for full-rate PE matmuls."""
    return ap.bitcast(FR)


def build_nc(E=1024, S_KV=2048, S_Q=1024, DFF=4096, n_rep=1):
    """Build the per-core Bass program.

    DRAM I/O (per core):
      src_kv [E, S_KV]  f32   src[b].T           (feature-major)
      src_q  [E, S_Q]   f32   src[b, half].T     (feature-major)
      Wq,Wk,Wv,Wo [E,E] f32 ; W1 [E,DFF] ; W2 [DFF,E]
      bq,bv,bo,g1,be1,g2,be2 [P, E//P] f32  (column j = vec[j*128:(j+1)*128])
      b1 [P, DFF//P] f32 ; b2 [P, E//P] f32
      out [E, S_Q] f32  (feature-major result)
    """
    ET = E // P          # feature tiles
    TT = S_KV // P       # kv token tiles
    NKV = S_KV // 512    # 512-wide moving blocks over kv tokens
    QB = S_Q // 256      # 256-query attention blocks
    HT = DFF // P        # ffn hidden tiles
    HC = HT // 8         # ffn hidden chunks (8 tiles = 1024 each)
    SC = S_Q // 512      # 512-wide moving blocks over queries
    EC = E // 256        # 256-wide moving blocks over features

    nc = bacc.Bacc(None, target_bir_lowering=False)

    src_kv = nc.dram_tensor("src_kv", [E, S_KV], FR, kind="ExternalInput")
    src_q = nc.dram_tensor("src_q", [E, S_Q], FR, kind="ExternalInput")
    Wq = nc.dram_tensor("Wq", [E, E], FR, kind="ExternalInput")
    Wk = nc.dram_tensor("Wk", [E, E], FR, kind="ExternalInput")
    Wv = nc.dram_tensor("Wv", [E, E], FR, kind="ExternalInput")
    Wo = nc.dram_tensor("Wo", [E, E], FR, kind="ExternalInput")
    W1 = nc.dram_tensor("W1", [E, DFF], FR, kind="ExternalInput")
    W2 = nc.dram_tensor("W2", [DFF, E], FR, kind="ExternalInput")
    bq = nc.dram_tensor("bq", [P, ET], FP, kind="ExternalInput")
    bv = nc.dram_tensor("bv", [P, ET], FP, kind="ExternalInput")
    bo = nc.dram_tensor("bo", [P, ET], FP, kind="ExternalInput")
    b1 = nc.dram_tensor("b1", [P, HT], FP, kind="ExternalInput")
    b2 = nc.dram_tensor("b2", [P, ET], FP, kind="ExternalInput")
    g1 = nc.dram_tensor("g1", [P, ET], FP, kind="ExternalInput")
    be1 = nc.dram_tensor("be1", [P, ET], FP, kind="ExternalInput")
    g2 = nc.dram_tensor("g2", [P, ET], FP, kind="ExternalInput")
    be2 = nc.dram_tensor("be2", [P, ET], FP, kind="ExternalInput")
    negones_d = nc.dram_tensor("negones", [P, 1], FR, kind="ExternalInput")
    out = nc.dram_tensor("out", [E, S_Q], FP, kind="ExternalOutput")

    def col_tiles(w, c0, c1, rows0=0, rows1=None):
        """DRAM view [rows, c0:c1] -> [P, rows//P, c1-c0] (feature-tile major)."""
        rows1 = w.shape[0] if rows1 is None else rows1
        return w[rows0:rows1, c0:c1].rearrange("(i p) m -> p i m", p=P)

    with tile.TileContext(nc) as tc:
        consts = tc.alloc_tile_pool(name="consts", bufs=1)
        bq_sb = consts.tile([P, ET], FP)
        bv_sb = consts.tile([P, ET], FP)
        bo_sb = consts.tile([P, ET], FP)
        b1_sb = consts.tile([P, HT], FP)
        b2_sb = consts.tile([P, ET], FP)
        g1_sb = consts.tile([P, ET], FP)
        be1_sb = consts.tile([P, ET], FP)
        g2_sb = consts.tile([P, ET], FP)
        be2_sb = consts.tile([P, ET], FP)
        for sb, dr in [(bq_sb, bq), (bv_sb, bv), (bo_sb, bo), (b1_sb, b1),
                       (b2_sb, b2), (g1_sb, g1), (be1_sb, be1), (g2_sb, g2),
                       (be2_sb, be2)]:
            nc.sync.dma_start(out=sb, in_=dr[:, :])
        ident = consts.tile([P, P], FP)
        make_identity(nc, ident)
        negones = consts.tile([P, 1], FR)
        nc.sync.dma_start(out=negones, in_=negones_d[:, :])
        eps_sb = consts.tile([1, 1], FP)
        nc.vector.memset(eps_sb, EPS)

        for _rep in range(n_rep):
            _body(nc, tc, locals())
        consts.release()
    nc.compile()
    return nc


def _body(nc, tc, g):
    """One full layer pass (separated so the whole body can be repeated
    for on-hardware timing runs)."""
    E, S_KV, S_Q = g["E"], g["S_KV"], g["S_Q"]
    ET, TT, NKV, QB, HT, HC, SC, EC = (g["ET"], g["TT"], g["NKV"], g["QB"],
                                       g["HT"], g["HC"], g["SC"], g["EC"])
    src_kv, src_q, out = g["src_kv"], g["src_q"], g["out"]
    Wq, Wk, Wv, Wo, W1, W2 = g["Wq"], g["Wk"], g["Wv"], g["Wo"], g["W1"], g["W2"]
    bq_sb, bv_sb, bo_sb, b1_sb, b2_sb = (g["bq_sb"], g["bv_sb"], g["bo_sb"],
                                         g["b1_sb"], g["b2_sb"])
    g1_sb, be1_sb, g2_sb, be2_sb = g["g1_sb"], g["be1_sb"], g["g2_sb"], g["be2_sb"]
    ident, negones, eps_sb = g["ident"], g["negones"], g["eps_sb"]
    col_tiles = g["col_tiles"]

    x_pool = tc.alloc_tile_pool(name="x", bufs=1)
    x_sb = x_pool.tile([P, ET, S_Q], FR)         # post-LN1 activations

    kv_pool = tc.alloc_tile_pool(name="kv", bufs=1)
    K_sb = kv_pool.tile([P, ET, S_KV], FR)
    V_sb = kv_pool.tile([P, TT, E], BF)          # token-major V, bf16

    # ---------------- Phase 1: K (feature-major) and V (token-major) -------
    # src is streamed in token-halves so only S_KV/2 columns are resident.
    NH = 2 if S_KV >= 2048 else 1
    SH = S_KV // NH          # tokens per half
    with tc.tile_pool(name="p1src", bufs=1) as srcp, \
            tc.tile_pool(name="p1w", bufs=2) as wp, \
            tc.tile_pool(name="p1ps", bufs=2, space="PSUM") as psp:
        for th in range(NH):
            t0 = th * SH
            skv = srcp.tile([P, ET, SH], FR, tag="skv")
            for ei in range(ET):
                nc.sync.dma_start(out=skv[:, ei, :],
                                  in_=src_kv[ei * P:(ei + 1) * P, t0:t0 + SH])

            # V[t, e] = (src.T @ Wv)[t, e]; lhsT = src_fm tile, moving = Wv
            for ec in range(EC):
                wv_blk = wp.tile([P, ET, 256], FR, tag="wv")
                nc.sync.dma_start(out=wv_blk,
                                  in_=col_tiles(Wv, ec * 256, (ec + 1) * 256))
                for tt in range(SH // P):
                    ps = psp.tile([P, 256], FP, tag="v")
                    for ei in range(ET):
                        nc.tensor.matmul(ps, _r(skv[:, ei, tt * P:(tt + 1) * P]),
                                         _r(wv_blk[:, ei, :]),
                                         start=(ei == 0), stop=(ei == ET - 1))
                    nc.vector.tensor_copy(
                        out=V_sb[:, t0 // P + tt, ec * 256:(ec + 1) * 256], in_=ps)

            # K_fm[eo, t] ; lhsT = Wk tile, moving = src_fm
            for eo in range(ET):
                wk_blk = wp.tile([P, ET, P], FR, tag="wk")
                nc.sync.dma_start(out=wk_blk,
                                  in_=col_tiles(Wk, eo * P, (eo + 1) * P))
                for tn in range(SH // 512):
                    ps = psp.tile([P, 512], FP, tag="k")
                    for ei in range(ET):
                        nc.tensor.matmul(
                            ps, _r(wk_blk[:, ei, :]),
                            _r(skv[:, ei, tn * 512:(tn + 1) * 512]),
                            start=(ei == 0), stop=(ei == ET - 1))
                    nc.vector.tensor_copy(
                        out=K_sb[:, eo, t0 + tn * 512:t0 + (tn + 1) * 512], in_=ps)

    # ---------------- Phase 2: attention + out-proj + LN1, 256 queries/blk -
    with tc.tile_pool(name="a_in", bufs=1) as ain, \
            tc.tile_pool(name="a_w", bufs=2) as awp, \
            tc.tile_pool(name="a_p", bufs=1) as app, \
            tc.tile_pool(name="a_small", bufs=1) as asm, \
            tc.tile_pool(name="a_psqs", bufs=2, space="PSUM") as ps_qs, \
            tc.tile_pool(name="a_pst", bufs=2, space="PSUM") as ps_t, \
            tc.tile_pool(name="a_psao", bufs=2, space="PSUM") as ps_ao, \
            tc.tile_pool(name="a_pso", bufs=1, space="PSUM") as ps_o, \
            tc.tile_pool(name="a_psst", bufs=1, space="PSUM") as ps_st:
        for blk in range(QB):
            q0 = blk * 256
            sq_blk = ain.tile([P, ET, 256], FR, tag="srcq")
            nc.sync.dma_start(out=sq_blk, in_=col_tiles(src_q, q0, q0 + 256))

            # Q projection (+bq)
            Q_blk = ain.tile([P, ET, 256], FR, tag="q")
            for eo in range(ET):
                wq_blk = awp.tile([P, ET, P], FR, tag="wq")
                nc.sync.dma_start(out=wq_blk, in_=col_tiles(Wq, eo * P, (eo + 1) * P))
                ps = ps_qs.tile([P, 512], FP, tag="qs")
                for ei in range(ET):
                    nc.tensor.matmul(ps[:, :256], _r(wq_blk[:, ei, :]),
                                     _r(sq_blk[:, ei, :]),
                                     start=(ei == 0), stop=(ei == ET - 1))
                nc.scalar.activation(out=Q_blk[:, eo, :], in_=ps[:, :256],
                                     func=ACTF.Identity, bias=bq_sb[:, eo:eo + 1],
                                     scale=1.0)

            # scores + softmax + fused normalize-transpose, per 128-query tile
            PT_blk = app.tile([P, TT, 256], BF, tag="pt", bufs=1)
            for st in range(2):
                P_sb = app.tile([P, S_KV], FP, tag="p")
                nmx = asm.tile([P, NKV], FP, tag="nmx")
                for tn in range(NKV):
                    ps = ps_qs.tile([P, 512], FP, tag="qs")
                    for ei in range(ET):
                        nc.tensor.matmul(
                            ps, _r(Q_blk[:, ei, st * P:(st + 1) * P]),
                            _r(K_sb[:, ei, tn * 512:(tn + 1) * 512]),
                            start=(ei == 0), stop=(ei == ET - 1))
                    nc.vector.reduce_max(out=nmx[:, tn:tn + 1], in_=ps,
                                         axis=AX.X, negate=True)
                    nc.vector.tensor_copy(out=P_sb[:, tn * 512:(tn + 1) * 512], in_=ps)
                negmax = asm.tile([P, 1], FP, tag="negmax")
                nc.vector.tensor_reduce(out=negmax, in_=nmx, op=ALU.min, axis=AX.X)
                sumexp = asm.tile([P, 1], FP, tag="sumexp")
                nc.scalar.activation(out=P_sb, in_=P_sb, func=ACTF.Exp,
                                     bias=negmax, scale=1.0, accum_out=sumexp)
                rsum = asm.tile([P, 1], FP, tag="rsum")
                nc.vector.reciprocal(out=rsum, in_=sumexp)
                # normalize rows in place, then transpose each 128x128 block
                nc.vector.tensor_scalar_mul(out=P_sb, in0=P_sb, scalar1=rsum)
                for tp in range(TT // 4):
                    ps = ps_t.tile([P, 512], FP, tag="t")
                    for k in range(4):
                        ttx = tp * 4 + k
                        nc.tensor.transpose(
                            ps[:, k * P:(k + 1) * P],
                            P_sb[:, ttx * P:(ttx + 1) * P], ident)
                    nc.vector.tensor_copy(
                        out=PT_blk[:, tp * 4:(tp + 1) * 4, st * P:(st + 1) * P],
                        in_=ps.rearrange("p (a b) -> p a b", a=4))

            # ao[e, s] = sum_t V[t, e] * PT[t, s]   (+bv)
            ao_blk = ain.tile([P, ET, 256], FR, tag="ao")
            for eo in range(ET):
                ps = ps_ao.tile([P, 256], FP, tag="ao")
                for tt in range(TT):
                    nc.tensor.matmul(ps, V_sb[:, tt, eo * P:(eo + 1) * P],
                                     PT_blk[:, tt, :],
                                     start=(tt == 0), stop=(tt == TT - 1))
                nc.scalar.activation(out=ao_blk[:, eo, :], in_=ps,
                                     func=ACTF.Identity, bias=bv_sb[:, eo:eo + 1],
                                     scale=1.0)

            # o = ao @ Wo + bo ; y1 = o + src_q (pre-LN1), written over sq_blk
            for eo in range(ET):
                wo_blk = awp.tile([P, ET, P], FR, tag="wo")
                nc.sync.dma_start(out=wo_blk, in_=col_tiles(Wo, eo * P, (eo + 1) * P))
                ps = ps_o.tile([P, 256], FP, tag="o")
                for ei in range(ET):
                    nc.tensor.matmul(ps, _r(wo_blk[:, ei, :]), _r(ao_blk[:, ei, :]),
                                     start=(ei == 0), stop=(ei == ET - 1))
                nc.vector.scalar_tensor_tensor(
                    out=sq_blk[:, eo, :], in0=ps, scalar=bo_sb[:, eo:eo + 1],
                    in1=sq_blk[:, eo, :], op0=ALU.add, op1=ALU.add)

            # LN1 -> x
            _layernorm(nc, tc, sq_blk, x_sb[:, :, q0:q0 + 256], 256, ET, E,
                       negones, eps_sb, g1_sb, be1_sb, ps_st, asm)

    kv_pool.release()

    # ---------------- Phase 3: FFN + LN2 -----------------------------------
    with tc.tile_pool(name="f_h", bufs=2) as fhp, \
            tc.tile_pool(name="f_ff", bufs=1) as ffp, \
            tc.tile_pool(name="f_w", bufs=2) as fwp, \
            tc.tile_pool(name="f_small", bufs=2) as fsm, \
            tc.tile_pool(name="f_psh", bufs=2, space="PSUM") as ps_h, \
            tc.tile_pool(name="f_psf", bufs=2, space="PSUM") as ps_f, \
            tc.tile_pool(name="f_psst", bufs=1, space="PSUM") as ps_st2:
        ff_sb = ffp.tile([P, ET, S_Q], FR)
        for chunk in range(HC):
            h_sb = fhp.tile([P, 8, S_Q], FR, tag="h")
            for hl in range(8):
                ht = chunk * 8 + hl
                w1_blk = fwp.tile([P, ET, P], FR, tag="w1")
                nc.sync.dma_start(out=w1_blk, in_=col_tiles(W1, ht * P, (ht + 1) * P))
                for sc in range(SC):
                    ps = ps_h.tile([P, 512], FP, tag="h")
                    for ei in range(ET):
                        nc.tensor.matmul(ps, _r(w1_blk[:, ei, :]),
                                         _r(x_sb[:, ei, sc * 512:(sc + 1) * 512]),
                                         start=(ei == 0), stop=(ei == ET - 1))
                    nc.scalar.activation(out=h_sb[:, hl, sc * 512:(sc + 1) * 512],
                                         in_=ps, func=ACTF.Relu,
                                         bias=b1_sb[:, ht:ht + 1], scale=1.0)
            for eo in range(ET):
                w2_blk = fwp.tile([P, 8, P], FR, tag="w2")
                nc.sync.dma_start(
                    out=w2_blk,
                    in_=col_tiles(W2, eo * P, (eo + 1) * P,
                                  rows0=chunk * 1024, rows1=(chunk + 1) * 1024))
                for sc in range(SC):
                    ps = ps_f.tile([P, 512], FP, tag="f")
                    for hl in range(8):
                        nc.tensor.matmul(ps, _r(w2_blk[:, hl, :]),
                                         _r(h_sb[:, hl, sc * 512:(sc + 1) * 512]),
                                         start=(hl == 0), stop=(hl == 7))
                    dst = ff_sb[:, eo, sc * 512:(sc + 1) * 512]
                    if chunk == 0:
                        nc.scalar.activation(out=dst, in_=ps, func=ACTF.Identity,
                                             bias=b2_sb[:, eo:eo + 1], scale=1.0)
                    else:
                        nc.vector.tensor_add(out=dst, in0=dst, in1=ps)

        # y2 = x + ff (in place into ff_sb) ; LN2 -> out
        with tc.tile_pool(name="f_out", bufs=1) as fop:
            for sc in range(SC):
                y2 = ff_sb[:, :, sc * 512:(sc + 1) * 512]
                for ei in range(ET):
                    nc.vector.tensor_add(out=y2[:, ei, :], in0=y2[:, ei, :],
                                         in1=x_sb[:, ei, sc * 512:(sc + 1) * 512])
                o_sb = fop.tile([P, ET, 512], FP, tag="out")
                _layernorm(nc, tc, y2, o_sb, 512, ET, E,
                           negones, eps_sb, g2_sb, be2_sb, ps_st2, fsm)
                for eo in range(ET):
                    nc.sync.dma_start(out=out[eo * P:(eo + 1) * P,
                                              sc * 512:(sc + 1) * 512],
                                      in_=o_sb[:, eo, :])
    x_pool.release()


def _layernorm(nc, tc, y_blk, x_dst, n, ET, E, negones, eps_sb, g_sb, be_sb,
               ps_pool, sm_pool):
    """Feature-axis layernorm on feature-major y_blk [P, ET, n] -> x_dst.

    Stats via ones-matmul over the partition axis (negones = -1/E), token-wise
    mean/rstd broadcast back across partitions with gpsimd, per-feature
    affine folded into the ACT evacuation.
    """
    ps = ps_pool.tile([1, 2 * n], FP, tag="st")
    sq = sm_pool.tile([P, n], FR, tag="lnsq")
    for ei in range(ET):
        nc.tensor.matmul(ps[0:1, 0:n], _r(negones), _r(y_blk[:, ei, :]),
                         start=(ei == 0), stop=(ei == ET - 1))
    for ei in range(ET):
        nc.scalar.activation(out=sq, in_=y_blk[:, ei, :], func=ACTF.Square,
                             bias=0.0, scale=1.0)
        nc.tensor.matmul(ps[0:1, n:2 * n], _r(negones), _r(sq),
                         start=(ei == 0), stop=(ei == ET - 1))
    negmu = sm_pool.tile([1, n], FP, tag="lnmu")
    nc.vector.tensor_copy(out=negmu, in_=ps[0:1, 0:n])
    var = sm_pool.tile([1, n], FP, tag="lnvar")
    # var = -negmeansq - negmu^2 = E[y^2] - mu^2
    sqmu = sm_pool.tile([1, n], FP, tag="lnsqmu")
    nc.vector.tensor_mul(out=sqmu, in0=negmu, in1=negmu)
    nc.vector.scalar_tensor_tensor(out=var, in0=ps[0:1, n:2 * n], scalar=-1.0,
                                   in1=sqmu, op0=ALU.mult, op1=ALU.subtract)
    nc.scalar.activation(out=var, in_=var, func=ACTF.Sqrt, bias=eps_sb[0:1, :],
                         scale=1.0)
    rs = sm_pool.tile([1, n], FP, tag="lnrs")
    nc.vector.reciprocal(out=rs, in_=var)
    negmu_bc = sm_pool.tile([P, n], FP, tag="lnmubc")
    rs_bc = sm_pool.tile([P, n], FP, tag="lnrsbc")
    nc.gpsimd.partition_broadcast(negmu_bc, negmu)
    nc.gpsimd.partition_broadcast(rs_bc, rs)
    t = sm_pool.tile([P, n], FP, tag="lnt")
    for ei in range(ET):
        nc.vector.tensor_add(out=t, in0=y_blk[:, ei, :], in1=negmu_bc)
        nc.vector.tensor_mul(out=t, in0=t, in1=rs_bc)
        nc.scalar.activation(out=x_dst[:, ei, :], in_=t, func=ACTF.Identity,
                             bias=be_sb[:, ei:ei + 1], scale=g_sb[:, ei:ei + 1])


# ---------------------------------------------------------------------------
# Host side: shard, run on 8 cores, gather.
# ---------------------------------------------------------------------------

_NC_CACHE = {}


def _get_nc():
    key = "full"
    if key not in _NC_CACHE:
        _NC_CACHE[key] = build_nc()
    return _NC_CACHE[key]


def _bias_cols(v, et):
    return np.ascontiguousarray(v.reshape(et, P).T)


def kernel(**inputs):
    from concourse.bass_utils import run_bass_kernel_spmd

    src = np.asarray(inputs["src"], dtype=np.float32)
    B, S, E = src.shape            # (4, 2048, 1024)
    half = S // 2
    ET, HT = E // P, inputs["W1"].shape[1] // P

    w = {k: np.ascontiguousarray(np.asarray(inputs[k], np.float32))
         for k in ("Wq", "Wk", "Wv", "Wo", "W1", "W2")}
    b = {k: _bias_cols(np.asarray(inputs[k], np.float32),
                       HT if k == "b1" else ET)
         for k in ("bq", "bv", "bo", "b1", "b2", "g1", "be1", "g2", "be2")}

    in_maps = []
    for c in range(8):
        bb, hh = c // 2, c % 2
        m = dict(w)
        m.update(b)
        m["negones"] = np.full((P, 1), -1.0 / E, np.float32)
        m["src_kv"] = np.ascontiguousarray(src[bb].T)
        m["src_q"] = np.ascontiguousarray(src[bb, hh * half:(hh + 1) * half, :].T)
        in_maps.append(m)

    nc = _get_nc()
    res = run_bass_kernel_spmd(nc, in_maps, core_ids=list(range(8)))

    y = np.empty((B, S, E), dtype=np.float32)
    for c in range(8):
        bb, hh = c // 2, c % 2
        y[bb, hh * half:(hh + 1) * half, :] = res.results[c]["out"].T
    return y

